# revision 1
# baseline (speedup 1.0000x reference)
"""Trainium2 Bass kernel for nn_ClauseInferModule (gnn_message_passing).

out[c, b, g] = sum_s prod_l x[b, I[c, g, s, l]],  B=16 G=16384 C=8 S=8 L=4.

Sharding: clause-per-core (C == n_cores == 8). Per core:
  - x is staged in SBUF as [128, G] f32 with partition p holding x[p % 16, :]
    (the batch replicated across the 8 GPSIMD core groups),
  - GPSIMD ap_gather pulls x[b, idx] for 16 b at once (idx shared across the
    16 partitions of a Q7 core group); each of the 8 groups processes its own
    2048-atom slice of the clause for one substitution s per call,
  - VectorE multiplies the L=4 gathered blocks (step-1 fp32 tensor_tensor)
    and accumulates over the S=8 calls,
  - one DMA writes the finished (B, G) clause slab to HBM.

The int64 index tensor is converted to the int16 "wrapped in 16 partitions"
ap_gather layout on the host (a pure dtype/layout transform).
"""
import os
import sys
import numpy as np

sys.path.insert(0, "/opt/trn_rl_repo")

import concourse.bacc as bacc
import concourse.tile as tile
from concourse import mybir
from concourse.bass_utils import run_bass_kernel_spmd

B, G = 16, 16384
C, S, L = 8, 8, 4
NIDX = 8192          # gathers per core group per ap_gather call (= 4l x 2048g)
GCHUNK = G // 8      # 2048 target atoms per core group

_compiled = None
last_exec_time_ns = None


def _build():
    nc = bacc.Bacc("TRN2", target_bir_lowering=False, debug=False)
    x_d = nc.dram_tensor("x", [B, G], mybir.dt.float32, kind="ExternalInput")
    idx_d = nc.dram_tensor("idx", [128, S * (NIDX // 16)], mybir.dt.int16,
                           kind="ExternalInput")
    out_d = nc.dram_tensor("out", [B, G], mybir.dt.float32, kind="ExternalOutput")

    with tile.TileContext(nc) as tc:
        with tc.tile_pool(name="xq", bufs=1) as xq, \
             tc.tile_pool(name="ip", bufs=2) as ip, \
             tc.tile_pool(name="gp", bufs=2) as gp, \
             tc.tile_pool(name="wp", bufs=1) as wp, \
             tc.tile_pool(name="aq", bufs=1) as aq:
            x_tile = xq.tile([128, G], mybir.dt.float32)
            for k in range(8):
                nc.sync.dma_start(out=x_tile[16 * k:16 * (k + 1), :],
                                  in_=x_d[:, :])
            acc = aq.tile([128, GCHUNK], mybir.dt.float32)
            for s in range(S):
                it = ip.tile([128, NIDX // 16], mybir.dt.int16, tag="it")
                nc.sync.dma_start(
                    out=it[:, :],
                    in_=idx_d[:, s * (NIDX // 16):(s + 1) * (NIDX // 16)])
                g = gp.tile([128, NIDX], mybir.dt.float32, tag="g")
                nc.gpsimd.ap_gather(g[:, :], x_tile[:, :], it[:, :],
                                    channels=128, num_elems=G, d=1,
                                    num_idxs=NIDX)

                def A(l):
                    return g[:, l * GCHUNK:(l + 1) * GCHUNK]

                tm1 = wp.tile([128, GCHUNK], mybir.dt.float32, tag="tm1")
                tm2 = wp.tile([128, GCHUNK], mybir.dt.float32, tag="tm2")
                nc.vector.tensor_mul(tm1[:, :], A(0), A(1))
                nc.vector.tensor_mul(tm2[:, :], A(2), A(3))
                if s == 0:
                    nc.vector.tensor_mul(acc[:, :], tm1[:, :], tm2[:, :])
                else:
                    tm3 = wp.tile([128, GCHUNK], mybir.dt.float32, tag="tm3")
                    nc.vector.tensor_mul(tm3[:, :], tm1[:, :], tm2[:, :])
                    nc.vector.tensor_add(acc[:, :], acc[:, :], tm3[:, :])
            # acc[16k + b, w] = out[b, k*2048 + w]
            for k in range(8):
                nc.sync.dma_start(
                    out=out_d[:, k * GCHUNK:(k + 1) * GCHUNK],
                    in_=acc[16 * k:16 * (k + 1), :])
    nc.compile()
    return nc


def _prep_idx(I: np.ndarray) -> np.ndarray:
    """[C, G, S, L] int64 -> [C, 128, S*512] int16 wrapped ap_gather feed.

    Call s of clause c: core group k gathers, at stream position
    i = l*2048 + w, the atom index I[c, k*2048 + w, s, l]. ap_gather reads
    position i of group k from it[16*k + i%16, i//16].
    """
    T = I.astype(np.int16).reshape(C, 8, GCHUNK, S, L)     # [c,k,w,s,l]
    U = T.transpose(0, 3, 1, 4, 2).reshape(C, S, 8, NIDX)  # [c,s,k,i=l*2048+w]
    W = U.reshape(C, S, 8, NIDX // 16, 16)                 # [c,s,k,col,pp]
    W = W.transpose(0, 2, 4, 1, 3)                         # [c,k,pp,s,col]
    return np.ascontiguousarray(W).reshape(C, 128, S * (NIDX // 16))


def kernel(x: np.ndarray, I: np.ndarray) -> np.ndarray:
    global _compiled, last_exec_time_ns
    if _compiled is None:
        _compiled = _build()
    nc = _compiled

    x = np.ascontiguousarray(np.asarray(x), dtype=np.float32)
    idx_feed = _prep_idx(np.asarray(I))

    in_maps = [{"x": x, "idx": idx_feed[c]} for c in range(C)]
    kwargs = {}
    if os.environ.get("KERNEL_TRACE") == "1":
        kwargs = {"trace": True, "trace_cores": list(range(C))}
    res = run_bass_kernel_spmd(nc, in_maps, core_ids=list(range(C)), **kwargs)
    last_exec_time_ns = res.exec_time_ns
    out = np.stack([res.results[c]["out"] for c in range(C)], axis=0)
    return np.ascontiguousarray(out, dtype=np.float32)


if __name__ == "__main__":
    rng = np.random.default_rng(0)
    x = rng.random((B, G), dtype=np.float32)
    I = rng.integers(0, G, size=(C, G, S, L)).astype(np.int64)
    out = kernel(x=x, I=I)
    gathered = x[:, I]
    expect = np.moveaxis(np.sum(np.prod(gathered, axis=-1), axis=-1), 0, 1)
    err = np.abs(out - expect).max() / np.abs(expect).max()
    print("max rel err:", err)



# revision 10
# speedup vs baseline: 1.7442x; 1.7442x over previous
"""Trainium2 Bass kernel for nn_ClauseInferModule (gnn_message_passing).

out[c, b, g] = sum_s prod_l x[b, I[c, g, s, l]],  B=16 G=16384 C=8 S=8 L=4.

Sharding: clause-per-core (C == n_cores == 8). Per core the 524288 random
lookups run on the SWDGE dma_gather path instead of GPSIMD ap_gather:

  - the valuation table lives in HBM as xt[a, 0:16] = x[:, a] padded to a
    256-byte row stride (elem_step=64 f32), so one descriptor fetches the
    16-batch vector of one atom (64B payload) directly into the consumer
    layout [128 positions, W cols, 16 b] -- no on-chip extraction pass,
  - 64 dma_gather calls of 8192 indices each (one per (s, g-half, l); 8192*4B
    of Q7 index scratch stays under the 64KB SCRATCH_BUF_SIZE). The Q7 core
    pair of SWDGE queue l emits descriptors while the 16 SDMA engines execute
    them, so gather throughput is descriptor-rate bound (~14x the ap_gather
    RD_CMD rate). single_packet=False: concatenating 512 descriptors into one
    SDMA packet exceeds the packet ceiling and wedges the device,
  - index tiles rotate through 8 slots; each [128, 2048] group tile holds
    the four l-calls' wrapped index lists (replicated across the 8 Q7 core
    groups, as the dma_gather contract requires),
  - VectorE multiplies the four l-streams and accumulates over s,
  - one DMA per g-half writes the [128, 64, 16] accumulator to HBM; the host
    undoes the (p, h, w, b) -> (b, g) layout.

dma_gather's bass wrapper insists on elem_size % 256B == 0 (a transpose-mode
restriction); the non-transpose ucode only needs the row *stride* 256B-aligned,
so the instruction is emitted directly with elem_size=16 f32 (64B payload).
"""
import os
import sys
import numpy as np

sys.path.insert(0, "/opt/trn_rl_repo")

import concourse.bacc as bacc
import concourse.bass as bass
from concourse import mybir
from concourse.bass_utils import run_bass_kernel_spmd
from concourse.library_config import mlp

B, G = 16, 16384
C, S, L = 8, 8, 4
W = 64                # columns per gather call
NIDX = 128 * W        # 8192 indices per dma_gather call (one (s, half, l))
H = 2                 # g-halves per s
NGROUP = S * H        # 16 (s, half) groups
NCALL = NGROUP * L    # 64 calls per core
NQ = 4                # SWDGE queues; queue == l
RSETS = 2             # rotating gather-buffer sets (s parity)
IDX_COLS = NIDX // 16      # 512 wrapped idx columns per call
GRP_COLS = L * IDX_COLS    # 2048 columns per (s, half) group tile
IDX_SLOTS = 8              # rotating group-tile slots

_compiled = None
last_exec_time_ns = None


def _emit_dma_gather(gp, out_ap, in_ap, idxs_ap, num_idxs, elem_size,
                     elem_step, queue_num):
    """mybir.InstDMAGatherAnt emit, mirroring BassGpSimd.dma_gather but
    without the transpose-only elem_size%256B restriction (the non-transpose
    ucode only requires the row stride to be a 256B multiple)."""
    stride_bytes = elem_step * mybir.dt.size(in_ap.dtype)
    assert stride_bytes % 256 == 0 and stride_bytes // 256 < 256
    assert in_ap.ap[0][0] == elem_step
    assert in_ap.ap[-1][1] == out_ap.ap[-1][1] == elem_size
    assert out_ap.ap[0][1] * out_ap.ap[1][1] == num_idxs
    assert num_idxs * 4 + 1024 < (1 << 16) - 64  # Q7 scratch buffer limit
    _in_ap = gp.lower_ap_dma(in_ap, for_custom_bir_dma=True)
    inst = gp.add_instruction(
        mybir.InstDMAGatherAnt(
            name=gp.bass.get_next_instruction_name(),
            ins=[
                *_in_ap,
                gp.lower_ap(idxs_ap),
                gp.lower_val_access(gp.to_reg(num_idxs)),
            ],
            outs=[gp.lower_ap(out_ap)],
            transpose=False,
            num_idxs=num_idxs,
            elem_size=elem_size,
            stride_bytes_256=stride_bytes // 256,
            gen_mode=0,
            single_packet=False,
            queue_num=queue_num,
        )
    )
    return inst


def _build():
    nc = bacc.Bacc("TRN2", target_bir_lowering=False, debug=False,
                   num_swdge_queues=NQ, dynamic_dma_scratch_size=32768)
    xt_d = nc.dram_tensor("xt", [G, 64], mybir.dt.float32,
                          kind="ExternalInput")
    idx_d = nc.dram_tensor("idx", [128, NGROUP * GRP_COLS], mybir.dt.int16,
                           kind="ExternalInput")
    out_d = nc.dram_tensor("out", [128, H, W, 16], mybir.dt.float32,
                           kind="ExternalOutput")

    from contextlib import ExitStack
    with ExitStack() as ctx:
        block = ctx.enter_context(nc.Block())
        bufs = [[[ctx.enter_context(
                     nc.sbuf_tensor(f"buf_{r}_{h}_{l}", [128, W, 16],
                                    mybir.dt.float32))
                  for l in range(L)] for h in range(H)] for r in range(RSETS)]
        idxt = [ctx.enter_context(
                    nc.sbuf_tensor(f"idx_{j}", [128, GRP_COLS],
                                   mybir.dt.int16))
                for j in range(IDX_SLOTS)]
        t1 = ctx.enter_context(
            nc.sbuf_tensor("t1", [128, W, 16], mybir.dt.float32))
        t2 = ctx.enter_context(
            nc.sbuf_tensor("t2", [128, W, 16], mybir.dt.float32))
        t3 = ctx.enter_context(
            nc.sbuf_tensor("t3", [128, W, 16], mybir.dt.float32))
        acc = [ctx.enter_context(
                   nc.sbuf_tensor(f"acc{h}", [128, W, 16], mybir.dt.float32))
               for h in range(H)]
        # One outstanding DMA per semaphore => cumulative waits are exact.
        idx_sem = [ctx.enter_context(nc.semaphore(f"idx_sem{j}"))
                   for j in range(IDX_SLOTS)]
        gat_sem = [[[ctx.enter_context(nc.semaphore(f"gat{q}_{r}_{h}"))
                     for h in range(H)] for r in range(RSETS)]
                   for q in range(NQ)]
        dve_sem = ctx.enter_context(nc.semaphore("dve_sem"))
        vchain = ctx.enter_context(nc.semaphore("vchain"))
        out_sem = ctx.enter_context(nc.semaphore("out_sem"))

        @block.sync
        def _(sync):
            for g in range(NGROUP):
                if g >= IDX_SLOTS:
                    # slot free once all 4 gathers of group g-IDX_SLOTS ran
                    gp_, hp = divmod(g - IDX_SLOTS, H)
                    for q in range(NQ):
                        sync.wait_ge(gat_sem[q][gp_ % RSETS][hp],
                                     16 * (gp_ // RSETS + 1))
                sync.dma_start(
                    idxt[g % IDX_SLOTS][:, :],
                    idx_d[:, g * GRP_COLS:(g + 1) * GRP_COLS],
                ).then_inc(idx_sem[g % IDX_SLOTS], 16)
            sync.wait_ge(dve_sem, NGROUP)
            for h in range(H):
                sync.dma_start(out_d[:, h, :, :], acc[h][:, :, :]) \
                    .then_inc(out_sem, 16)
            sync.wait_ge(out_sem, 16 * H)

        @block.gpsimd
        def _(gp):
            gp.load_library(mlp)
            for k in range(NCALL):
                g, l = divmod(k, L)
                s, h = divmod(g, H)
                r = s % RSETS
                if l == 0:
                    gp.wait_ge(idx_sem[g % IDX_SLOTS],
                               16 * (g // IDX_SLOTS + 1))
                    if s >= RSETS:
                        # buffer set free once DVE consumed group (s-RSETS, h)
                        gp.wait_ge(dve_sem, (s - RSETS) * H + h + 1)
                _emit_dma_gather(
                    gp,
                    out_ap=bufs[r][h][l][:, :, :],
                    in_ap=xt_d[:, 0:16],
                    idxs_ap=idxt[g % IDX_SLOTS][:,
                        l * IDX_COLS:(l + 1) * IDX_COLS],
                    num_idxs=NIDX,
                    elem_size=16,
                    elem_step=64,
                    queue_num=l,
                ).then_inc(gat_sem[l][r][h], 16)

        @block.vector
        def _(vec):
            # DVE executes in order, but raw-block mode has no implicit
            # dependency tracking: serialize the stream through vchain /
            # dve_sem (one sem update per instruction).
            nv, nd = 0, 0

            def op(final, f, *args):
                nonlocal nv, nd
                if nv:
                    vec.wait_ge(vchain, nv)
                if nd:
                    vec.wait_ge(dve_sem, nd)
                inst = f(*args)
                if final:
                    inst.then_inc(dve_sem, 1)
                    nd += 1
                else:
                    inst.then_inc(vchain, 1)
                    nv += 1
                return inst

            for g in range(NGROUP):
                s, h = divmod(g, H)
                r = s % RSETS
                for q in range(NQ):
                    vec.wait_ge(gat_sem[q][r][h], 16 * (s // RSETS + 1))
                v = bufs[r][h]
                a = acc[h]
                op(0, vec.tensor_mul, t1[:, :, :], v[0][:, :, :], v[1][:, :, :])
                op(0, vec.tensor_mul, t2[:, :, :], v[2][:, :, :], v[3][:, :, :])
                if s == 0:
                    op(1, vec.tensor_mul, a[:, :, :], t1[:, :, :], t2[:, :, :])
                else:
                    op(0, vec.tensor_mul, t3[:, :, :], t1[:, :, :], t2[:, :, :])
                    op(1, vec.tensor_add, a[:, :, :], a[:, :, :], t3[:, :, :])

    nc.compile()
    return nc


def _prep_inputs(x: np.ndarray, I: np.ndarray):
    """Host-side layout transforms: padded transposed table + wrapped int16
    index streams. Group tile (s, h) packs the four l-calls' index lists in
    32-partition bands (each band: wrapped-in-16 layout, duplicated for the
    queue's TX and RX Q7 cores)."""
    xt = np.zeros((G, 64), np.float32)
    xt[:, 0:16] = x.T
    feeds = []
    for c in range(C):
        arr = np.empty((128, NGROUP * GRP_COLS), np.int16)
        for g in range(NGROUP):
            s, h = divmod(g, H)
            for l in range(L):
                v = I[c, h * NIDX:(h + 1) * NIDX, s, l].astype(np.int16)
                w = v.reshape(IDX_COLS, 16).T          # wrapped [16, 512]
                arr[:, g * GRP_COLS + l * IDX_COLS:
                    g * GRP_COLS + (l + 1) * IDX_COLS] = np.tile(w, (8, 1))
        feeds.append(arr)
    return xt, feeds


def kernel(x: np.ndarray, I: np.ndarray) -> np.ndarray:
    global _compiled, last_exec_time_ns
    if _compiled is None:
        _compiled = _build()
    nc = _compiled

    x = np.ascontiguousarray(np.asarray(x), dtype=np.float32)
    xt, idx_feeds = _prep_inputs(x, np.asarray(I))

    in_maps = [{"xt": xt, "idx": idx_feeds[c]} for c in range(C)]
    kwargs = {}
    if os.environ.get("KERNEL_TRACE") == "1":
        kwargs = {"trace": True, "trace_cores": list(range(C))}
    res = run_bass_kernel_spmd(nc, in_maps, core_ids=list(range(C)), **kwargs)
    last_exec_time_ns = res.exec_time_ns
    # res[c]["out"][p, h, w, b] = out[c, b, h*8192 + w*128 + p]
    out = np.empty((C, B, G), np.float32)
    for c in range(C):
        r = np.asarray(res.results[c]["out"]).reshape(128, H, W, 16)
        out[c] = r.transpose(3, 1, 2, 0).reshape(B, G)
    return np.ascontiguousarray(out)


if __name__ == "__main__":
    rng = np.random.default_rng(0)
    x = rng.random((B, G), dtype=np.float32)
    I = rng.integers(0, G, size=(C, G, S, L)).astype(np.int64)
    out = kernel(x=x, I=I)
    gathered = x[:, I]
    expect = np.moveaxis(np.sum(np.prod(gathered, axis=-1), axis=-1), 0, 1)
    err = np.abs(out - expect).max() / np.abs(expect).max()
    print("max rel err:", err)


# revision 11
# speedup vs baseline: 1.7806x; 1.0209x over previous
"""Trainium2 Bass kernel for nn_ClauseInferModule (gnn_message_passing).

out[c, b, g] = sum_s prod_l x[b, I[c, g, s, l]],  B=16 G=16384 C=8 S=8 L=4.

Sharding: clause-per-core (C == n_cores == 8). Per core the 524288 random
lookups run on the SWDGE dma_gather path instead of GPSIMD ap_gather:

  - the valuation table lives in HBM as xt[a, 0:16] = x[:, a] padded to a
    256-byte row stride (elem_step=64 f32), so one descriptor fetches the
    16-batch vector of one atom (64B payload) directly into the consumer
    layout [128 positions, W cols, 16 b] -- no on-chip extraction pass,
  - 64 dma_gather calls of 8192 indices each (one per (s, g-half, l); 8192*4B
    of Q7 index scratch stays under the 64KB SCRATCH_BUF_SIZE). The Q7 core
    pair of SWDGE queue l emits descriptors while the 16 SDMA engines execute
    them, so gather throughput is descriptor-rate bound (~14x the ap_gather
    RD_CMD rate). single_packet=False: concatenating 512 descriptors into one
    SDMA packet exceeds the packet ceiling and wedges the device,
  - index tiles rotate through 8 slots; each [128, 2048] group tile holds
    the four l-calls' wrapped index lists (replicated across the 8 Q7 core
    groups, as the dma_gather contract requires),
  - VectorE multiplies the four l-streams and accumulates over s,
  - one DMA per g-half writes the [128, 64, 16] accumulator to HBM; the host
    undoes the (p, h, w, b) -> (b, g) layout.

dma_gather's bass wrapper insists on elem_size % 256B == 0 (a transpose-mode
restriction); the non-transpose ucode only needs the row *stride* 256B-aligned,
so the instruction is emitted directly with elem_size=16 f32 (64B payload).
"""
import os
import sys
import numpy as np

sys.path.insert(0, "/opt/trn_rl_repo")

import concourse.bacc as bacc
import concourse.bass as bass
from concourse import mybir
from concourse.bass_utils import run_bass_kernel_spmd
from concourse.library_config import mlp

B, G = 16, 16384
C, S, L = 8, 8, 4
W = 64                # columns per gather call
NIDX = 128 * W        # 8192 indices per dma_gather call (one (s, half, l))
H = 2                 # g-halves per s
NGROUP = S * H        # 16 (s, half) groups
NCALL = NGROUP * L    # 64 calls per core
NQ = 4                # SWDGE queues; queue == l
RSETS = 2             # rotating gather-buffer sets (s parity)
IDX_COLS = NIDX // 16      # 512 wrapped idx columns per call
GRP_COLS = L * IDX_COLS    # 2048 columns per (s, half) group tile
IDX_SLOTS = 8              # rotating group-tile slots

_compiled = None
last_exec_time_ns = None


def _emit_dma_gather(gp, out_ap, in_ap, idxs_ap, num_idxs, elem_size,
                     elem_step, queue_num, src_sbuf=False):
    """mybir.InstDMAGatherAnt emit, mirroring BassGpSimd.dma_gather but
    without the transpose-only elem_size%256B restriction and allowing an
    SBUF source in non-transpose mode (the ucode's gen_descs handles both;
    only the bass wrapper and the interp restrict them)."""
    assert in_ap.ap[-1][1] == out_ap.ap[-1][1] == elem_size
    assert out_ap.ap[0][1] * out_ap.ap[1][1] == num_idxs
    assert num_idxs * 4 + 1024 < (1 << 16) - 64  # Q7 scratch buffer limit
    if src_sbuf:
        stride_bytes_256 = 0
        sbuf_kw = dict(sbuf_tokens_per_rank=128,
                       sbuf_free_dim_per_rank=elem_size * 4,
                       sbuf_free_dim_pad_per_rank=0, sbuf_byte_offset=0)
        _in_ap = [gp.lower_ap(in_ap)]
    else:
        stride_bytes = elem_step * mybir.dt.size(in_ap.dtype)
        assert stride_bytes % 256 == 0 and stride_bytes // 256 < 256
        assert in_ap.ap[0][0] == elem_step
        stride_bytes_256 = stride_bytes // 256
        sbuf_kw = {}
        _in_ap = gp.lower_ap_dma(in_ap, for_custom_bir_dma=True)
    inst = gp.add_instruction(
        mybir.InstDMAGatherAnt(
            name=gp.bass.get_next_instruction_name(),
            ins=[
                *_in_ap,
                gp.lower_ap(idxs_ap),
                gp.lower_val_access(gp.to_reg(num_idxs)),
            ],
            outs=[gp.lower_ap(out_ap)],
            transpose=False,
            num_idxs=num_idxs,
            elem_size=elem_size,
            stride_bytes_256=stride_bytes_256,
            gen_mode=0,
            single_packet=False,
            queue_num=queue_num,
            **sbuf_kw,
        )
    )
    return inst


def _build(src_sbuf: bool = False):
    nc = bacc.Bacc("TRN2", target_bir_lowering=False, debug=False,
                   num_swdge_queues=NQ, dynamic_dma_scratch_size=32768)
    if src_sbuf:
        xt_d = nc.dram_tensor("xt", [128, G // 128, 16], mybir.dt.float32,
                              kind="ExternalInput")
    else:
        xt_d = nc.dram_tensor("xt", [G, 64], mybir.dt.float32,
                              kind="ExternalInput")
    idx_d = nc.dram_tensor("idx", [128, NGROUP * GRP_COLS], mybir.dt.int16,
                           kind="ExternalInput")
    out_d = nc.dram_tensor("out", [128, H, W, 16], mybir.dt.float32,
                           kind="ExternalOutput")

    from contextlib import ExitStack
    with ExitStack() as ctx:
        block = ctx.enter_context(nc.Block())
        bufs = [[[ctx.enter_context(
                     nc.sbuf_tensor(f"buf_{r}_{h}_{l}", [128, W, 16],
                                    mybir.dt.float32))
                  for l in range(L)] for h in range(H)] for r in range(RSETS)]
        idxt = [ctx.enter_context(
                    nc.sbuf_tensor(f"idx_{j}", [128, GRP_COLS],
                                   mybir.dt.int16))
                for j in range(IDX_SLOTS)]
        t1 = ctx.enter_context(
            nc.sbuf_tensor("t1", [128, W, 16], mybir.dt.float32))
        t2 = ctx.enter_context(
            nc.sbuf_tensor("t2", [128, W, 16], mybir.dt.float32))
        t3 = ctx.enter_context(
            nc.sbuf_tensor("t3", [128, W, 16], mybir.dt.float32))
        acc = [ctx.enter_context(
                   nc.sbuf_tensor(f"acc{h}", [128, W, 16], mybir.dt.float32))
               for h in range(H)]
        xs = (ctx.enter_context(
                  nc.sbuf_tensor("xs", [128, G // 128, 16], mybir.dt.float32))
              if src_sbuf else None)
        # One outstanding DMA per semaphore => cumulative waits are exact.
        idx_sem = [ctx.enter_context(nc.semaphore(f"idx_sem{j}"))
                   for j in range(IDX_SLOTS)]
        gat_sem = [[[ctx.enter_context(nc.semaphore(f"gat{q}_{r}_{h}"))
                     for h in range(H)] for r in range(RSETS)]
                   for q in range(NQ)]
        dve_sem = ctx.enter_context(nc.semaphore("dve_sem"))
        vchain = ctx.enter_context(nc.semaphore("vchain"))
        out_sem = ctx.enter_context(nc.semaphore("out_sem"))
        xs_sem = ctx.enter_context(nc.semaphore("xs_sem"))

        @block.sync
        def _(sync):
            if src_sbuf:
                sync.dma_start(xs[:, :, :], xt_d[:, :, :]).then_inc(xs_sem, 16)
            for g in range(NGROUP):
                if g >= IDX_SLOTS:
                    # slot free once all 4 gathers of group g-IDX_SLOTS ran
                    gp_, hp = divmod(g - IDX_SLOTS, H)
                    for q in range(NQ):
                        sync.wait_ge(gat_sem[q][gp_ % RSETS][hp],
                                     16 * (gp_ // RSETS + 1))
                sync.dma_start(
                    idxt[g % IDX_SLOTS][:, :],
                    idx_d[:, g * GRP_COLS:(g + 1) * GRP_COLS],
                ).then_inc(idx_sem[g % IDX_SLOTS], 16)
            sync.wait_ge(dve_sem, NGROUP)
            for h in range(H):
                sync.dma_start(out_d[:, h, :, :], acc[h][:, :, :]) \
                    .then_inc(out_sem, 16)
            sync.wait_ge(out_sem, 16 * H)

        @block.gpsimd
        def _(gp):
            gp.load_library(mlp)
            if src_sbuf:
                gp.wait_ge(xs_sem, 16)
            for k in range(NCALL):
                g, l = divmod(k, L)
                s, h = divmod(g, H)
                r = s % RSETS
                if l == 0:
                    gp.wait_ge(idx_sem[g % IDX_SLOTS],
                               16 * (g // IDX_SLOTS + 1))
                    if s >= RSETS:
                        # buffer set free once DVE consumed group (s-RSETS, h)
                        gp.wait_ge(dve_sem, (s - RSETS) * H + h + 1)
                _emit_dma_gather(
                    gp,
                    out_ap=bufs[r][h][l][:, :, :],
                    in_ap=xs[:, :, :] if src_sbuf else xt_d[:, 0:16],
                    idxs_ap=idxt[g % IDX_SLOTS][:,
                        l * IDX_COLS:(l + 1) * IDX_COLS],
                    num_idxs=NIDX,
                    elem_size=16,
                    elem_step=64,
                    queue_num=l,
                    src_sbuf=src_sbuf,
                ).then_inc(gat_sem[l][r][h], 16)

        @block.vector
        def _(vec):
            # DVE executes in order, but raw-block mode has no implicit
            # dependency tracking: serialize the stream through vchain /
            # dve_sem (one sem update per instruction).
            nv, nd = 0, 0

            def op(final, f, *args):
                nonlocal nv, nd
                if nv:
                    vec.wait_ge(vchain, nv)
                if nd:
                    vec.wait_ge(dve_sem, nd)
                inst = f(*args)
                if final:
                    inst.then_inc(dve_sem, 1)
                    nd += 1
                else:
                    inst.then_inc(vchain, 1)
                    nv += 1
                return inst

            for g in range(NGROUP):
                s, h = divmod(g, H)
                r = s % RSETS
                for q in range(NQ):
                    vec.wait_ge(gat_sem[q][r][h], 16 * (s // RSETS + 1))
                v = bufs[r][h]
                a = acc[h]
                op(0, vec.tensor_mul, t1[:, :, :], v[0][:, :, :], v[1][:, :, :])
                op(0, vec.tensor_mul, t2[:, :, :], v[2][:, :, :], v[3][:, :, :])
                if s == 0:
                    op(1, vec.tensor_mul, a[:, :, :], t1[:, :, :], t2[:, :, :])
                else:
                    op(0, vec.tensor_mul, t3[:, :, :], t1[:, :, :], t2[:, :, :])
                    op(1, vec.tensor_add, a[:, :, :], a[:, :, :], t3[:, :, :])

    nc.compile()
    return nc


def _prep_inputs(x: np.ndarray, I: np.ndarray, src_sbuf: bool = False):
    """Host-side layout transforms: padded transposed table + wrapped int16
    index streams. Group tile (s, h) packs the four l-calls' index lists in
    32-partition bands (each band: wrapped-in-16 layout, duplicated for the
    queue's TX and RX Q7 cores)."""
    if src_sbuf:
        # xs[p, r, b] = x[b, 128*r + p]
        xt = np.ascontiguousarray(
            np.transpose(x.reshape(B, G // 128, 128), (2, 1, 0)))
    else:
        xt = np.zeros((G, 64), np.float32)
        xt[:, 0:16] = x.T
    feeds = []
    for c in range(C):
        arr = np.empty((128, NGROUP * GRP_COLS), np.int16)
        for g in range(NGROUP):
            s, h = divmod(g, H)
            for l in range(L):
                v = I[c, h * NIDX:(h + 1) * NIDX, s, l].astype(np.int16)
                w = v.reshape(IDX_COLS, 16).T          # wrapped [16, 512]
                arr[:, g * GRP_COLS + l * IDX_COLS:
                    g * GRP_COLS + (l + 1) * IDX_COLS] = np.tile(w, (8, 1))
        feeds.append(arr)
    return xt, feeds


SRC_SBUF = os.environ.get("GATHER_SRC", "sbuf") == "sbuf"


def kernel(x: np.ndarray, I: np.ndarray) -> np.ndarray:
    global _compiled, last_exec_time_ns
    if _compiled is None:
        _compiled = _build(src_sbuf=SRC_SBUF)
    nc = _compiled

    x = np.ascontiguousarray(np.asarray(x), dtype=np.float32)
    xt, idx_feeds = _prep_inputs(x, np.asarray(I), src_sbuf=SRC_SBUF)

    in_maps = [{"xt": xt, "idx": idx_feeds[c]} for c in range(C)]
    kwargs = {}
    if os.environ.get("KERNEL_TRACE") == "1":
        kwargs = {"trace": True, "trace_cores": list(range(C))}
    res = run_bass_kernel_spmd(nc, in_maps, core_ids=list(range(C)), **kwargs)
    last_exec_time_ns = res.exec_time_ns
    # res[c]["out"][p, h, w, b] = out[c, b, h*8192 + w*128 + p]
    out = np.empty((C, B, G), np.float32)
    for c in range(C):
        r = np.asarray(res.results[c]["out"]).reshape(128, H, W, 16)
        out[c] = r.transpose(3, 1, 2, 0).reshape(B, G)
    return np.ascontiguousarray(out)


if __name__ == "__main__":
    rng = np.random.default_rng(0)
    x = rng.random((B, G), dtype=np.float32)
    I = rng.integers(0, G, size=(C, G, S, L)).astype(np.int64)
    out = kernel(x=x, I=I)
    gathered = x[:, I]
    expect = np.moveaxis(np.sum(np.prod(gathered, axis=-1), axis=-1), 0, 1)
    err = np.abs(out - expect).max() / np.abs(expect).max()
    print("max rel err:", err)
